# revision 1
# baseline (speedup 1.0000x reference)
"""MultiHeadDecoder (moe_routing) Trainium2 kernel.

Strategy: expert-parallel. Each of the 8 cores owns one head's weights.
Host groups samples by head index, pads each group to a common capacity C
(multiple of 64), and transposes X so the contraction dim lands on
partitions. Each core runs a dense 2-layer MLP (256->512 relu, 512->2048)
for its head's samples. Host scatters rows back to original order.

Layer 1 computes H^T (hid on partitions) so layer 2 can contract over hid
without an on-chip transpose:
  H^T[hc]  = W1[:, hc].T @ X^T      (lhsT=W1 chunk, rhs=X^T chunk)
  out[st]  = (H^T[:, st]).T @ W2    (lhsT=H^T chunk, rhs=W2 chunk)

Matmuls run in float32r (fp32 bits, full PE rate, tf32-ish multiply).
Inputs are packed host-side into the exact SBUF layout so every DMA has
long contiguous runs per partition. All inputs stream on the sync (SP)
HWDGE ring; all output stores go on the scalar (Act) ring so they never
queue behind the W2 stream (rings are FIFO per issuing engine). Stage B
is ordered oc-outer so only the first W2 chunk's DMA gates its start.
Dummy matmuls keep the PE's HAM clock-gate warm while DMAs stream.
"""

import numpy as np

import concourse.bass as bass
import concourse.mybir as mybir
from concourse import bacc
from concourse.tile import TileContext
from concourse.bass_utils import run_bass_kernel_spmd

IN_F, HID, OUT_F, N_HEADS, BATCH = 256, 512, 2048, 8, 4096
N_CORES = 8
P = 128
KI = IN_F // P     # 2  input-feature chunks
HC = HID // P      # 4  hidden chunks
OC = OUT_F // 512  # 4  output-feature chunks of 512

f32 = mybir.dt.float32
f32r = mybir.dt.float32r  # fp32 bits, PE runs at full (bf16) rate, tf32-ish mul

_NC_CACHE: dict = {}


def build_nc(C: int):
    """Build the per-core Bass program for capacity C (multiple of 64)."""
    KF = C + HID     # free size of one k-part: xt_k then w1_k
    stiles = [(s, min(P, C - s)) for s in range(0, C, P)]
    sgroups = [(s, min(512, C - s)) for s in range(0, C, 512)]

    nc = bacc.Bacc("TRN2", target_bir_lowering=False, debug=False,
                   num_devices=N_CORES)
    xin = nc.dram_tensor("xin", [KI, P, KF], f32r, kind="ExternalInput")
    b1s = nc.dram_tensor("b1s", [P, HC], f32, kind="ExternalInput")
    w2p = nc.dram_tensor("w2p", [OC, P, HC * 512], f32r, kind="ExternalInput")
    b2 = nc.dram_tensor("b2", [1, OUT_F], f32, kind="ExternalInput")
    out = nc.dram_tensor("out", [C, OUT_F], f32, kind="ExternalOutput")

    relu = mybir.ActivationFunctionType.Relu

    with TileContext(nc) as tc:
        with (
            tc.tile_pool(name="const", bufs=1) as const,
            tc.tile_pool(name="psumA", bufs=2, space="PSUM") as psumA,
            tc.tile_pool(name="psumB", bufs=5, space="PSUM") as psumB,
            tc.tile_pool(name="psumW", bufs=1, space="PSUM") as psumW,
            tc.tile_pool(name="outp", bufs=6) as outp,
        ):
            # HAM warmup: dummy matmuls with no DMA deps keep the PE busy
            # while inputs stream in, so real matmuls run at 2.4 GHz.
            wsrc = const.tile([P, 64], f32, tag="warm")
            nc.vector.memset(wsrc[:], 0.0)
            wps = psumW.tile([64, 64], f32, tag="warmps")
            for _ in range(35):
                nc.tensor.matmul(wps[:], lhsT=wsrc[:, :64], rhs=wsrc[:],
                                 start=True, stop=True)

            # Stage-A inputs first so the PE starts ASAP; W2 streams behind.
            # Two k-part DMAs so the k=0 matmuls can start at half-arrival.
            xin_ks = []
            for k in range(KI):
                xk = const.tile([P, KF], f32r, tag=f"xin_{k}")
                nc.sync.dma_start(xk[:], xin[k])
                xin_ks.append(xk)
            b1_s = const.tile([P, HC], f32)
            nc.sync.dma_start(b1_s[:], b1s[:])
            b2_row = const.tile([1, OUT_F], f32)
            nc.sync.dma_start(b2_row[:], b2[:])
            b2_s = const.tile([P, OUT_F], f32)
            nc.gpsimd.partition_broadcast(b2_s[:], b2_row[:])
            w2_cs = []
            for oc in range(OC):
                w2_c = const.tile([P, HC * 512], f32r, tag=f"w2_{oc}")
                nc.sync.dma_start(w2_c[:], w2p[oc])
                w2_cs.append(w2_c)

            # Stage A: H^T [hid(part), sample(free)], relu(x @ W1 + b1)
            # sgroups outer so stage B's early sample tiles are ready sooner.
            ht = const.tile([P, HC, C], f32r)
            for (s0, sn) in sgroups:
                for hc in range(HC):
                    ps = psumA.tile([P, 512], f32, tag="psA")
                    for k in range(KI):
                        nc.tensor.matmul(
                            ps[:, :sn],
                            lhsT=xin_ks[k][:, C + hc * P: C + (hc + 1) * P],
                            rhs=xin_ks[k][:, s0:s0 + sn],
                            start=(k == 0), stop=(k == KI - 1),
                        )
                    nc.scalar.activation(
                        ht[:, hc, s0:s0 + sn], ps[:, :sn], relu,
                        bias=b1_s[:, hc:hc + 1],
                    )

            # Bridge warmup: keep the PE hot while the first W2 chunk lands.
            for _ in range(12):
                nc.tensor.matmul(wps[:], lhsT=wsrc[:, :64], rhs=wsrc[:],
                                 start=True, stop=True)

            # Stage B: out[st, oc] = H[st] @ W2[:, oc] + b2[oc]
            for oc in range(OC):
                for (s0, sn) in stiles:
                    ps = psumB.tile([P, 512], f32, tag="psB")
                    for hc in range(HC):
                        nc.tensor.matmul(
                            ps[:sn, :],
                            lhsT=ht[:, hc, s0:s0 + sn],
                            rhs=w2_cs[oc][:, hc * 512:(hc + 1) * 512],
                            start=(hc == 0), stop=(hc == HC - 1),
                        )
                    ot = outp.tile([P, 512], f32, tag="ot")
                    nc.vector.tensor_add(
                        out=ot[:sn, :],
                        in0=ps[:sn, :],
                        in1=b2_s[:sn, oc * 512:(oc + 1) * 512],
                    )
                    nc.scalar.dma_start(
                        out[s0:s0 + sn, oc * 512:(oc + 1) * 512], ot[:sn, :]
                    )

    nc.compile()
    return nc


def kernel(X, X_head_idx, W1, b1, W2, b2):
    X = np.ascontiguousarray(np.asarray(X, dtype=np.float32))
    idx = np.asarray(X_head_idx).astype(np.int64)
    W1 = np.asarray(W1, dtype=np.float32)
    b1 = np.asarray(b1, dtype=np.float32)
    W2 = np.asarray(W2, dtype=np.float32)
    b2 = np.asarray(b2, dtype=np.float32)

    batch = X.shape[0]
    counts = np.bincount(idx, minlength=N_HEADS)
    order = np.argsort(idx, kind="stable")
    positions = np.split(order, np.cumsum(counts)[:-1])

    C = max(512, int(-(-counts.max() // 64)) * 64)
    if C not in _NC_CACHE:
        _NC_CACHE[C] = build_nc(C)
    nc = _NC_CACHE[C]

    in_maps = []
    for h in range(N_HEADS):
        pos = positions[h]
        # xin[k, p, :] = [ X[pos, k*128+p] (len C, padded) | W1[h, k*128+p, :] ]
        xin = np.zeros((KI, P, C + HID), dtype=np.float32)
        if len(pos):
            xk = X[pos].T.reshape(KI, P, len(pos))          # [k, p, c]
            xin[:, :, :len(pos)] = xk
        xin[:, :, C:] = W1[h].reshape(KI, P, HID)
        # w2 packed: [oc, p, hc*512 + o'] = W2[h, hc*128 + p, oc*512 + o']
        w2t = np.transpose(W2[h].reshape(HC, P, OUT_F), (1, 0, 2))  # [p, hc, of]
        w2p = np.empty((OC, P, HC * 512), dtype=np.float32)
        for oc in range(OC):
            w2p[oc] = w2t[:, :, oc * 512:(oc + 1) * 512].reshape(P, HC * 512)
        in_maps.append({
            "xin": xin,
            "b1s": np.ascontiguousarray(b1[h].reshape(HC, P).T),
            "w2p": w2p,
            "b2": np.ascontiguousarray(b2[h][None, :]),
        })

    try:
        res = run_bass_kernel_spmd(nc, in_maps, list(range(N_CORES)))
    except Exception:
        res = run_bass_kernel_spmd(nc, in_maps, list(range(N_CORES)))

    out = np.empty((batch, OUT_F), dtype=np.float32)
    for h in range(N_HEADS):
        pos = positions[h]
        if len(pos):
            out[pos] = res.results[h]["out"][:len(pos)]
    return out



# revision 11
# speedup vs baseline: 1.0319x; 1.0319x over previous
"""MultiHeadDecoder (moe_routing) Trainium2 kernel, v3.

Expert-parallel: each of 8 cores owns one head. Host groups samples by
head, pads to capacity C (multiple of 8), ships everything bf16 (PSUM
accumulates f32; tolerance 2e-2 vs bf16 wire error ~1e-3).

Both stages keep weights stationary in the PE and stream sample columns,
so PE time tracks real sample count:
  stage A:  ht[hc][hid,s]  = relu(sum_k W1[k,hc]^T @ X^T[k][:,s] + b1)
  stage B:  outT[of][of,s] = sum_hc W2[of,hc]^T @ ht[hc][:,s] + b2
Output is transposed ([out_feature, sample]); host untransposes.

DMA efficiency is driven by per-partition contiguous line length, so
inputs are packed into few transfers with 2-16KB lines:
  xin  [P, KI*C + KI*HID + HC + OF]  (X^T | W1 | b1 | b2), bf16
  w2   [4, P, 2048]  (4 of-tiles per block, line 4KB)
Outputs go out in of-pairs (line 4*C bytes) over 3 rings.

Sample columns split into two groups of G=C/2 (<=512/psum bank); bias
adds alternate ACT (activation+bias) / DVE (tensor_scalar add).
Warmup matmuls cover the input-DMA window so HAM reaches full duty
before real work; trailing dummies keep it hot through the epilogue
semaphore storm (which otherwise runs at half clock).
"""

import numpy as np

import concourse.bass as bass
import concourse.mybir as mybir
from concourse import bacc
from concourse.tile import TileContext
from concourse.bass_utils import run_bass_kernel_spmd

IN_F, HID, OUT_F, N_HEADS, BATCH = 256, 512, 2048, 8, 4096
N_CORES = 8
P = 128
KI = IN_F // P      # 2 input-feature chunks
HC = HID // P       # 4 hidden chunks
OF = OUT_F // P     # 16 output-feature tiles
OB = 4              # of-tiles per W2 input DMA block

f32 = mybir.dt.float32
bf16 = mybir.dt.bfloat16

try:
    from ml_dtypes import bfloat16 as np_bf16
except ImportError:
    import jax.numpy as jnp
    np_bf16 = jnp.bfloat16

_NC_CACHE: dict = {}

WARM_PRE = 14    # 512-col warmups covering the input-DMA window
WARM_POST = 10   # trailing 512-col dummies keeping HAM hot into teardown


def build_nc(C: int):
    """Per-core Bass program for sample capacity C (multiple of 8)."""
    G = C // 2
    assert G <= 512
    XW = KI * C          # xin cols holding X^T
    WW = KI * HID        # xin cols holding W1
    NIN = XW + WW + HC + OF

    nc = bacc.Bacc("TRN2", target_bir_lowering=False, debug=False,
                   num_devices=N_CORES)
    xin = nc.dram_tensor("xin", [P, NIN], bf16, kind="ExternalInput")
    w2 = nc.dram_tensor("w2", [OF // OB, P, OB * HC * P], bf16,
                        kind="ExternalInput")
    outT = nc.dram_tensor("outT", [OF // 2, P, 2 * C], bf16,
                          kind="ExternalOutput")

    relu = mybir.ActivationFunctionType.Relu
    ident = mybir.ActivationFunctionType.Identity

    with TileContext(nc) as tc:
        with (
            tc.tile_pool(name="const", bufs=1) as const,
            tc.tile_pool(name="psumA", bufs=3, space="PSUM") as psumA,
            tc.tile_pool(name="psumB", bufs=4, space="PSUM") as psumB,
            tc.tile_pool(name="psumW", bufs=1, space="PSUM") as psumW,
            tc.tile_pool(name="outp", bufs=6) as outp,
        ):
            # Warmup matmuls on an uninitialized tile (values irrelevant).
            wsrc = const.tile([P, 512], bf16, tag="warm")
            nc.gpsimd.memset(wsrc[:, :1], 0.0)
            wps = psumW.tile([P, 512], f32, tag="warmps")
            for _ in range(WARM_PRE):
                nc.tensor.matmul(wps[:], lhsT=wsrc[:, :P], rhs=wsrc[:],
                                 start=True, stop=True)

            # --- input DMAs ---
            # gpsimd ring: xin (stage-A inputs, needed first)
            # sync ring:   w2 blocks 0-1; scalar ring: w2 blocks 2-3
            xs = const.tile([P, NIN], bf16, tag="xin")
            nc.gpsimd.dma_start(xs[:], xin[:])
            w2s = []
            for blk in range(OF // OB):
                t = const.tile([P, OB * HC * P], bf16, tag=f"w2_{blk}",
                               name=f"w2_{blk}")
                eng = nc.sync if blk < 2 else nc.scalar
                eng.dma_start(t[:], w2[blk])
                w2s.append(t)

            def xt_cols(k, g):
                base = k * C + g * G
                return xs[:, base:base + G]

            def w1_tile(k, hc):
                base = XW + k * HID + hc * P
                return xs[:, base:base + P]

            # biases ship as bf16 inside xin; convert once to f32 on-chip
            bconv = const.tile([P, HC + OF], f32, tag="bconv")
            nc.vector.tensor_scalar_add(bconv[:], xs[:, XW + WW:XW + WW + HC + OF], 0.0)
            b1_s = bconv[:, 0:HC]
            b2_s = bconv[:, HC:HC + OF]

            def w2_tile(of, hc):
                blk, j = divmod(of, OB)
                base = j * HC * P + hc * P
                return w2s[blk][:, base:base + P]

            # --- stage A: ht[hc] = relu(X @ W1 + b1)^T, bf16 ---
            hts = [const.tile([P, C], bf16, tag=f"ht{hc}", name=f"ht{hc}")
                   for hc in range(HC)]
            for hc in range(HC):
                pss = [psumA.tile([P, G], f32, tag="psA", name=f"psA{hc}_{g}")
                       for g in range(2)]
                for k in range(KI):
                    for g in range(2):
                        nc.tensor.matmul(
                            pss[g][:],
                            lhsT=w1_tile(k, hc),
                            rhs=xt_cols(k, g),
                            start=(k == 0), stop=(k == KI - 1),
                        )
                for g in range(2):
                    nc.scalar.activation(
                        hts[hc][:, g * G:(g + 1) * G], pss[g][:], relu,
                        bias=b1_s[:, hc:hc + 1],
                    )

            # --- stage B: outT[of] = (H @ W2 + b2)^T, bf16 ---
            out_rings = [nc.sync, nc.scalar, nc.gpsimd]
            ot = None
            for of in range(OF):
                pss = [psumB.tile([P, G], f32, tag="psB", name=f"psB{of}_{g}")
                       for g in range(2)]
                for hc in range(HC):
                    for g in range(2):
                        nc.tensor.matmul(
                            pss[g][:],
                            lhsT=w2_tile(of, hc),
                            rhs=hts[hc][:, g * G:(g + 1) * G],
                            start=(hc == 0), stop=(hc == HC - 1),
                        )
                if of % 2 == 0:
                    ot = outp.tile([P, 2 * C], bf16, tag="ot")
                off = (of % 2) * C
                for g in range(2):
                    dst = ot[:, off + g * G:off + (g + 1) * G]
                    if of % 2 == 0:
                        nc.scalar.activation(dst, pss[g][:], ident,
                                             bias=b2_s[:, of:of + 1])
                    else:
                        nc.vector.tensor_scalar_add(dst, pss[g][:],
                                                    b2_s[:, of:of + 1])
                if of % 2 == 1:
                    out_rings[(of // 2) % 3].dma_start(outT[of // 2], ot[:])

            # Trailing dummies: keep HAM at full duty through teardown.
            for _ in range(WARM_POST):
                nc.tensor.matmul(wps[:], lhsT=wsrc[:, :P], rhs=wsrc[:],
                                 start=True, stop=True)

    nc.compile()
    return nc


def kernel(X, X_head_idx, W1, b1, W2, b2):
    X = np.ascontiguousarray(np.asarray(X, dtype=np.float32))
    idx = np.asarray(X_head_idx).astype(np.int64)
    W1 = np.asarray(W1, dtype=np.float32)
    b1 = np.asarray(b1, dtype=np.float32)
    W2 = np.asarray(W2, dtype=np.float32)
    b2 = np.asarray(b2, dtype=np.float32)

    batch = X.shape[0]
    counts = np.bincount(idx, minlength=N_HEADS)
    order = np.argsort(idx, kind="stable")
    positions = np.split(order, np.cumsum(counts)[:-1])

    C = max(64, int(-(-int(counts.max()) // 8)) * 8)
    if C not in _NC_CACHE:
        _NC_CACHE[C] = build_nc(C)
    nc = _NC_CACHE[C]

    XW = KI * C
    WW = KI * HID
    NIN = XW + WW + HC + OF

    in_maps = []
    for h in range(N_HEADS):
        pos = positions[h]
        cnt = len(pos)
        xinf = np.zeros((P, NIN), dtype=np.float32)
        if cnt:
            # xin[p, k*C + s] = X[pos[s], k*128 + p]
            xk = X[pos].T.reshape(KI, P, cnt)  # [k, p, s]
            for k in range(KI):
                xinf[:, k * C:k * C + cnt] = xk[k]
        # xin[p, XW + k*HID + hcol] = W1[k*128 + p, hcol]
        w1r = W1[h].reshape(KI, P, HID)
        for k in range(KI):
            xinf[:, XW + k * HID:XW + (k + 1) * HID] = w1r[k]
        xinf[:, XW + WW:XW + WW + HC] = b1[h].reshape(HC, P).T
        xinf[:, XW + WW + HC:] = b2[h].reshape(OF, P).T
        # w2 packed: [blk, p, j*HC*P + hc*P + oc] = W2[hc*128+p, (blk*OB+j)*128+oc]
        w2r = W2[h].reshape(HC, P, OF, P)              # [hc, p, of, oc]
        w2p = np.transpose(w2r, (2, 1, 0, 3))          # [of, p, hc, oc]
        w2p = w2p.reshape(OF // OB, OB, P, HC * P)     # [blk, j, p, hc*oc]
        w2p = np.ascontiguousarray(np.transpose(w2p, (0, 2, 1, 3)))
        w2p = w2p.reshape(OF // OB, P, OB * HC * P)
        in_maps.append({
            "xin": xinf.astype(np_bf16),
            "w2": w2p.astype(np_bf16),
        })

    try:
        res = run_bass_kernel_spmd(nc, in_maps, list(range(N_CORES)))
    except Exception:
        res = run_bass_kernel_spmd(nc, in_maps, list(range(N_CORES)))

    out = np.empty((batch, OUT_F), dtype=np.float32)
    for h in range(N_HEADS):
        pos = positions[h]
        cnt = len(pos)
        if cnt:
            o = np.asarray(res.results[h]["outT"]).astype(np.float32)
            o = o.reshape(OF // 2, P, 2, C)            # [pair, p, half, s]
            o = np.transpose(o, (3, 0, 2, 1))          # [s, pair, half, p]
            o = o.reshape(C, OUT_F)
            out[pos] = o[:cnt]
    return out


# revision 15
# speedup vs baseline: 1.1418x; 1.1065x over previous
"""MultiHeadDecoder (moe_routing) Trainium2 kernel, v3.

Expert-parallel: each of 8 cores owns one head. Host groups samples by
head, pads to capacity C (multiple of 8), ships everything bf16 (PSUM
accumulates f32; tolerance 2e-2 vs bf16 wire error ~1e-3).

Both stages keep weights stationary in the PE and stream sample columns,
so PE time tracks real sample count:
  stage A:  ht[hc][hid,s]  = relu(sum_k W1[k,hc]^T @ X^T[k][:,s] + b1)
  stage B:  outT[of][of,s] = sum_hc W2[of,hc]^T @ ht[hc][:,s] + b2
Output is transposed ([out_feature, sample]); host untransposes.

DMA efficiency is driven by per-partition contiguous line length, so
inputs are packed into few transfers with 2-16KB lines:
  xin  [P, KI*C + KI*HID + HC + OF]  (X^T | W1 | b1 | b2), bf16
  w2   [4, P, 2048]  (4 of-tiles per block, line 4KB)
Outputs go out in of-pairs (line 4*C bytes) over 3 rings.

Sample columns split into two groups of G=C/2 (<=512/psum bank); bias
adds alternate ACT (activation+bias) / DVE (tensor_scalar add).
Warmup matmuls cover the input-DMA window so HAM reaches full duty
before real work; trailing dummies keep it hot through the epilogue
semaphore storm (which otherwise runs at half clock).
"""

import numpy as np

import concourse.bass as bass
import concourse.mybir as mybir
from concourse import bacc
from concourse.tile import TileContext
from concourse.bass_utils import run_bass_kernel_spmd

IN_F, HID, OUT_F, N_HEADS, BATCH = 256, 512, 2048, 8, 4096
N_CORES = 8
P = 128
KI = IN_F // P      # 2 input-feature chunks
HC = HID // P       # 4 hidden chunks
OF = OUT_F // P     # 16 output-feature tiles
OB = 4              # of-tiles per W2 input DMA block

f32 = mybir.dt.float32
bf16 = mybir.dt.bfloat16

try:
    from ml_dtypes import bfloat16 as np_bf16
except ImportError:
    import jax.numpy as jnp
    np_bf16 = jnp.bfloat16

_NC_CACHE: dict = {}

WARM_PRE = 5     # 512-col warmups covering the input-DMA window
WARM_MID = 3     # bridge dummies between stage A and stage B
WARM_POST = 14   # trailing 512-col dummies keeping HAM hot into teardown


def build_nc(C: int):
    """Per-core Bass program for sample capacity C (multiple of 8)."""
    G = C // 2
    assert G <= 512
    XW = KI * C          # xin cols holding X^T
    WW = KI * HID        # xin cols holding W1
    NIN = XW + WW + HC + OF

    nc = bacc.Bacc("TRN2", target_bir_lowering=False, debug=False,
                   num_devices=N_CORES)
    xin = nc.dram_tensor("xin", [P, NIN], bf16, kind="ExternalInput")
    w2 = nc.dram_tensor("w2", [OF // OB, P, OB * HC * P], bf16,
                        kind="ExternalInput")
    outT = nc.dram_tensor("outT", [OF // 2, P, 2 * C], bf16,
                          kind="ExternalOutput")

    relu = mybir.ActivationFunctionType.Relu
    ident = mybir.ActivationFunctionType.Identity

    with TileContext(nc) as tc:
        with (
            tc.tile_pool(name="const", bufs=1) as const,
            tc.tile_pool(name="psumA", bufs=3, space="PSUM") as psumA,
            tc.tile_pool(name="psumB", bufs=4, space="PSUM") as psumB,
            tc.tile_pool(name="psumW", bufs=1, space="PSUM") as psumW,
            tc.tile_pool(name="outp", bufs=6) as outp,
        ):
            # Warmup matmuls on an uninitialized tile (values irrelevant).
            wsrc = const.tile([P, 512], bf16, tag="warm")
            nc.gpsimd.memset(wsrc[:, :1], 0.0)
            wps = psumW.tile([P, 512], f32, tag="warmps")
            for _ in range(WARM_PRE):
                nc.tensor.matmul(wps[:], lhsT=wsrc[:, :P], rhs=wsrc[:],
                                 start=True, stop=True)

            # --- input DMAs ---
            # Per-ring bandwidth is ~150GB/s, so spread critical inputs:
            # xin halves on sync+scalar (stage A gates on it), W2 blocks
            # ordered by stage-B need across gpsimd/sync/scalar.
            xs = const.tile([P, NIN], bf16, tag="xin")
            H1 = NIN // 2
            nc.sync.dma_start(xs[:, :H1], xin[:, :H1])
            nc.scalar.dma_start(xs[:, H1:], xin[:, H1:])
            w2s = [const.tile([P, OB * HC * P], bf16, tag=f"w2_{blk}",
                              name=f"w2_{blk}") for blk in range(OF // OB)]
            nc.gpsimd.dma_start(w2s[0][:], w2[0])
            nc.sync.dma_start(w2s[1][:], w2[1])
            nc.scalar.dma_start(w2s[2][:], w2[2])
            nc.gpsimd.dma_start(w2s[3][:], w2[3])

            def xt_cols(k, g):
                base = k * C + g * G
                return xs[:, base:base + G]

            def w1_tile(k, hc):
                base = XW + k * HID + hc * P
                return xs[:, base:base + P]

            # biases ship as bf16 inside xin; convert once to f32 on-chip
            bconv = const.tile([P, HC + OF], f32, tag="bconv")
            nc.vector.tensor_scalar_add(bconv[:], xs[:, XW + WW:XW + WW + HC + OF], 0.0)
            b1_s = bconv[:, 0:HC]
            b2_s = bconv[:, HC:HC + OF]

            def w2_tile(of, hc):
                blk, j = divmod(of, OB)
                base = j * HC * P + hc * P
                return w2s[blk][:, base:base + P]

            # --- stage A: ht[hc] = relu(X @ W1 + b1)^T, bf16 ---
            hts = [const.tile([P, C], bf16, tag=f"ht{hc}", name=f"ht{hc}")
                   for hc in range(HC)]
            for hc in range(HC):
                pss = [psumA.tile([P, G], f32, tag="psA", name=f"psA{hc}_{g}")
                       for g in range(2)]
                for k in range(KI):
                    for g in range(2):
                        nc.tensor.matmul(
                            pss[g][:],
                            lhsT=w1_tile(k, hc),
                            rhs=xt_cols(k, g),
                            start=(k == 0), stop=(k == KI - 1),
                        )
                for g in range(2):
                    nc.scalar.activation(
                        hts[hc][:, g * G:(g + 1) * G], pss[g][:], relu,
                        bias=b1_s[:, hc:hc + 1],
                    )

            # Bridge dummies: cover any PE gap while w2 block 0 lands.
            for _ in range(WARM_MID):
                nc.tensor.matmul(wps[:, :264], lhsT=wsrc[:, :P],
                                 rhs=wsrc[:, :264], start=True, stop=True)

            # --- stage B: outT[of] = (H @ W2 + b2)^T, bf16 ---
            out_rings = [nc.sync, nc.scalar, nc.gpsimd]
            ot = None
            for of in range(OF):
                pss = [psumB.tile([P, G], f32, tag="psB", name=f"psB{of}_{g}")
                       for g in range(2)]
                for hc in range(HC):
                    for g in range(2):
                        nc.tensor.matmul(
                            pss[g][:],
                            lhsT=w2_tile(of, hc),
                            rhs=hts[hc][:, g * G:(g + 1) * G],
                            start=(hc == 0), stop=(hc == HC - 1),
                        )
                if of % 2 == 0:
                    ot = outp.tile([P, 2 * C], bf16, tag="ot")
                off = (of % 2) * C
                for g in range(2):
                    dst = ot[:, off + g * G:off + (g + 1) * G]
                    if of % 2 == 0:
                        nc.scalar.activation(dst, pss[g][:], ident,
                                             bias=b2_s[:, of:of + 1])
                    else:
                        nc.vector.tensor_scalar_add(dst, pss[g][:],
                                                    b2_s[:, of:of + 1])
                # Last pair ships as two half transfers so the final (fully
                # exposed) DMA is half the size; earlier tiles go in pairs.
                if of == OF - 2:
                    out_rings[(of // 2) % 3].dma_start(
                        outT[of // 2][:, :C], ot[:, :C])
                elif of == OF - 1:
                    out_rings[(of // 2 + 1) % 3].dma_start(
                        outT[of // 2][:, C:], ot[:, C:])
                elif of % 2 == 1:
                    out_rings[(of // 2) % 3].dma_start(outT[of // 2], ot[:])

            # Trailing dummies: keep HAM at full duty through teardown.
            for _ in range(WARM_POST):
                nc.tensor.matmul(wps[:], lhsT=wsrc[:, :P], rhs=wsrc[:],
                                 start=True, stop=True)

    nc.compile()
    return nc


def kernel(X, X_head_idx, W1, b1, W2, b2):
    X = np.ascontiguousarray(np.asarray(X, dtype=np.float32))
    idx = np.asarray(X_head_idx).astype(np.int64)
    W1 = np.asarray(W1, dtype=np.float32)
    b1 = np.asarray(b1, dtype=np.float32)
    W2 = np.asarray(W2, dtype=np.float32)
    b2 = np.asarray(b2, dtype=np.float32)

    batch = X.shape[0]
    counts = np.bincount(idx, minlength=N_HEADS)
    order = np.argsort(idx, kind="stable")
    positions = np.split(order, np.cumsum(counts)[:-1])

    C = max(64, int(-(-int(counts.max()) // 8)) * 8)
    if C not in _NC_CACHE:
        _NC_CACHE[C] = build_nc(C)
    nc = _NC_CACHE[C]

    XW = KI * C
    WW = KI * HID
    NIN = XW + WW + HC + OF

    in_maps = []
    for h in range(N_HEADS):
        pos = positions[h]
        cnt = len(pos)
        xinf = np.zeros((P, NIN), dtype=np.float32)
        if cnt:
            # xin[p, k*C + s] = X[pos[s], k*128 + p]
            xk = X[pos].T.reshape(KI, P, cnt)  # [k, p, s]
            for k in range(KI):
                xinf[:, k * C:k * C + cnt] = xk[k]
        # xin[p, XW + k*HID + hcol] = W1[k*128 + p, hcol]
        w1r = W1[h].reshape(KI, P, HID)
        for k in range(KI):
            xinf[:, XW + k * HID:XW + (k + 1) * HID] = w1r[k]
        xinf[:, XW + WW:XW + WW + HC] = b1[h].reshape(HC, P).T
        xinf[:, XW + WW + HC:] = b2[h].reshape(OF, P).T
        # w2 packed: [blk, p, j*HC*P + hc*P + oc] = W2[hc*128+p, (blk*OB+j)*128+oc]
        w2r = W2[h].reshape(HC, P, OF, P)              # [hc, p, of, oc]
        w2p = np.transpose(w2r, (2, 1, 0, 3))          # [of, p, hc, oc]
        w2p = w2p.reshape(OF // OB, OB, P, HC * P)     # [blk, j, p, hc*oc]
        w2p = np.ascontiguousarray(np.transpose(w2p, (0, 2, 1, 3)))
        w2p = w2p.reshape(OF // OB, P, OB * HC * P)
        in_maps.append({
            "xin": xinf.astype(np_bf16),
            "w2": w2p.astype(np_bf16),
        })

    try:
        res = run_bass_kernel_spmd(nc, in_maps, list(range(N_CORES)))
    except Exception:
        res = run_bass_kernel_spmd(nc, in_maps, list(range(N_CORES)))

    out = np.empty((batch, OUT_F), dtype=np.float32)
    for h in range(N_HEADS):
        pos = positions[h]
        cnt = len(pos)
        if cnt:
            o = np.asarray(res.results[h]["outT"]).astype(np.float32)
            o = o.reshape(OF // 2, P, 2, C)            # [pair, p, half, s]
            o = np.transpose(o, (3, 0, 2, 1))          # [s, pair, half, p]
            o = o.reshape(C, OUT_F)
            out[pos] = o[:cnt]
    return out


# revision 20
# speedup vs baseline: 1.1974x; 1.0487x over previous
"""MultiHeadDecoder (moe_routing) Trainium2 kernel, v3.

Expert-parallel: each of 8 cores owns one head. Host groups samples by
head, pads to capacity C (multiple of 8), ships everything bf16 (PSUM
accumulates f32; tolerance 2e-2 vs bf16 wire error ~1e-3).

Both stages keep weights stationary in the PE and stream sample columns,
so PE time tracks real sample count:
  stage A:  ht[hc][hid,s]  = relu(sum_k W1[k,hc]^T @ X^T[k][:,s] + b1)
  stage B:  outT[of][of,s] = sum_hc W2[of,hc]^T @ ht[hc][:,s] + b2
Output is transposed ([out_feature, sample]); host untransposes.

DMA efficiency is driven by per-partition contiguous line length, so
inputs are packed into few transfers with 2-16KB lines:
  xin  [P, KI*C + KI*HID + HC + OF]  (X^T | W1 | b1 | b2), bf16
  w2   [4, P, 2048]  (4 of-tiles per block, line 4KB)
Outputs go out in of-pairs (line 4*C bytes) over 3 rings.

Sample columns split into two groups of G=C/2 (<=512/psum bank); bias
adds alternate ACT (activation+bias) / DVE (tensor_scalar add).
Warmup matmuls cover the input-DMA window so HAM reaches full duty
before real work; trailing dummies keep it hot through the epilogue
semaphore storm (which otherwise runs at half clock).
"""

import numpy as np

import concourse.bass as bass
import concourse.mybir as mybir
from concourse import bacc
from concourse.tile import TileContext
from concourse.bass_utils import run_bass_kernel_spmd

IN_F, HID, OUT_F, N_HEADS, BATCH = 256, 512, 2048, 8, 4096
N_CORES = 8
P = 128
KI = IN_F // P      # 2 input-feature chunks
HC = HID // P       # 4 hidden chunks
OF = OUT_F // P     # 16 output-feature tiles
OB = 4              # of-tiles per W2 input DMA block

f32 = mybir.dt.float32
bf16 = mybir.dt.bfloat16

try:
    from ml_dtypes import bfloat16 as np_bf16
except ImportError:
    import jax.numpy as jnp
    np_bf16 = jnp.bfloat16

_NC_CACHE: dict = {}

WARM_PRE = 11    # 264-col warmups covering the input-DMA window
WARM_MID = 2     # bridge dummies between stage A and stage B
WARM_POST = 16   # trailing 264-col dummies keeping HAM hot into teardown


def build_nc(C: int):
    """Per-core Bass program for sample capacity C (multiple of 8)."""
    G = C // 2
    assert G <= 512
    XW = KI * C          # xin cols holding X^T
    WW = KI * HID        # xin cols holding W1
    NIN = XW + WW + HC + OF

    nc = bacc.Bacc("TRN2", target_bir_lowering=False, debug=False,
                   num_devices=N_CORES)
    xin = nc.dram_tensor("xin", [P, NIN], bf16, kind="ExternalInput")
    w2 = nc.dram_tensor("w2", [OF // OB, P, OB * HC * P], bf16,
                        kind="ExternalInput")
    outT = nc.dram_tensor("outT", [OF // 2, P, 2 * C], bf16,
                          kind="ExternalOutput")

    relu = mybir.ActivationFunctionType.Relu
    ident = mybir.ActivationFunctionType.Identity

    with TileContext(nc) as tc:
        with (
            tc.tile_pool(name="const", bufs=1) as const,
            tc.tile_pool(name="psumA", bufs=3, space="PSUM") as psumA,
            tc.tile_pool(name="psumB", bufs=4, space="PSUM") as psumB,
            tc.tile_pool(name="psumW", bufs=1, space="PSUM") as psumW,
            tc.tile_pool(name="outp", bufs=6) as outp,
        ):
            # Warmup matmuls on an uninitialized tile (values irrelevant).
            wsrc = const.tile([P, 512], bf16, tag="warm")
            nc.gpsimd.memset(wsrc[:, :1], 0.0)
            wps = psumW.tile([P, 512], f32, tag="warmps")
            for _ in range(WARM_PRE):
                nc.tensor.matmul(wps[:, :264], lhsT=wsrc[:, :P],
                                 rhs=wsrc[:, :264], start=True, stop=True)

            # --- input DMAs ---
            # Per-ring bandwidth is ~150GB/s, so spread critical inputs:
            # xin halves on sync+scalar (stage A gates on it), W2 blocks
            # ordered by stage-B need across gpsimd/sync/scalar.
            xs = const.tile([P, NIN], bf16, tag="xin")
            H1 = NIN // 2
            nc.sync.dma_start(xs[:, :H1], xin[:, :H1])
            nc.scalar.dma_start(xs[:, H1:], xin[:, H1:])
            w2s = [const.tile([P, OB * HC * P], bf16, tag=f"w2_{blk}",
                              name=f"w2_{blk}") for blk in range(OF // OB)]
            # block 0 split in halves right behind the xin halves so stage B
            # can start the moment stage A drains; the rest ordered by need.
            HB = OB * HC * P // 2
            nc.sync.dma_start(w2s[0][:, :HB], w2[0][:, :HB])
            nc.scalar.dma_start(w2s[0][:, HB:], w2[0][:, HB:])
            nc.gpsimd.dma_start(w2s[1][:], w2[1])
            nc.sync.dma_start(w2s[2][:], w2[2])
            nc.scalar.dma_start(w2s[3][:], w2[3])

            def xt_cols(k, g):
                base = k * C + g * G
                return xs[:, base:base + G]

            def w1_tile(k, hc):
                base = XW + k * HID + hc * P
                return xs[:, base:base + P]

            # biases ship as bf16 inside xin; convert once to f32 on-chip
            bconv = const.tile([P, HC + OF], f32, tag="bconv")
            nc.vector.tensor_scalar_add(bconv[:], xs[:, XW + WW:XW + WW + HC + OF], 0.0)
            b1_s = bconv[:, 0:HC]
            b2_s = bconv[:, HC:HC + OF]

            def w2_tile(of, hc):
                blk, j = divmod(of, OB)
                base = j * HC * P + hc * P
                return w2s[blk][:, base:base + P]

            # --- stage A: ht[hc] = relu(X @ W1 + b1)^T, bf16 ---
            hts = [const.tile([P, C], bf16, tag=f"ht{hc}", name=f"ht{hc}")
                   for hc in range(HC)]
            for hc in range(HC):
                pss = [psumA.tile([P, G], f32, tag="psA", name=f"psA{hc}_{g}")
                       for g in range(2)]
                for k in range(KI):
                    for g in range(2):
                        nc.tensor.matmul(
                            pss[g][:],
                            lhsT=w1_tile(k, hc),
                            rhs=xt_cols(k, g),
                            start=(k == 0), stop=(k == KI - 1),
                        )
                for g in range(2):
                    nc.scalar.activation(
                        hts[hc][:, g * G:(g + 1) * G], pss[g][:], relu,
                        bias=b1_s[:, hc:hc + 1],
                    )

            # Bridge dummies: cover any PE gap while w2 block 0 lands.
            for _ in range(WARM_MID):
                nc.tensor.matmul(wps[:, :264], lhsT=wsrc[:, :P],
                                 rhs=wsrc[:, :264], start=True, stop=True)

            # --- stage B: outT[of] = (H @ W2 + b2)^T, bf16 ---
            out_rings = [nc.gpsimd, nc.sync, nc.scalar]
            ot = None
            for of in range(OF):
                pss = [psumB.tile([P, G], f32, tag="psB", name=f"psB{of}_{g}")
                       for g in range(2)]
                for hc in range(HC):
                    for g in range(2):
                        nc.tensor.matmul(
                            pss[g][:],
                            lhsT=w2_tile(of, hc),
                            rhs=hts[hc][:, g * G:(g + 1) * G],
                            start=(hc == 0), stop=(hc == HC - 1),
                        )
                if of % 2 == 0:
                    ot = outp.tile([P, 2 * C], bf16, tag="ot")
                off = (of % 2) * C
                for g in range(2):
                    dst = ot[:, off + g * G:off + (g + 1) * G]
                    if of % 2 == 0:
                        nc.scalar.activation(dst, pss[g][:], ident,
                                             bias=b2_s[:, of:of + 1])
                    else:
                        nc.vector.tensor_scalar_add(dst, pss[g][:],
                                                    b2_s[:, of:of + 1])
                # Last pair ships as two half transfers so the final (fully
                # exposed) DMA is half the size; earlier tiles go in pairs.
                if of == OF - 2:
                    out_rings[(of // 2) % 3].dma_start(
                        outT[of // 2][:, :C], ot[:, :C])
                elif of == OF - 1:
                    out_rings[(of // 2 + 1) % 3].dma_start(
                        outT[of // 2][:, C:], ot[:, C:])
                elif of % 2 == 1:
                    out_rings[(of // 2) % 3].dma_start(outT[of // 2], ot[:])

            # Trailing dummies: keep HAM at full duty through teardown.
            for _ in range(WARM_POST):
                nc.tensor.matmul(wps[:, :264], lhsT=wsrc[:, :P],
                                 rhs=wsrc[:, :264], start=True, stop=True)

    nc.compile()
    return nc


def kernel(X, X_head_idx, W1, b1, W2, b2):
    X = np.ascontiguousarray(np.asarray(X, dtype=np.float32))
    idx = np.asarray(X_head_idx).astype(np.int64)
    W1 = np.asarray(W1, dtype=np.float32)
    b1 = np.asarray(b1, dtype=np.float32)
    W2 = np.asarray(W2, dtype=np.float32)
    b2 = np.asarray(b2, dtype=np.float32)

    batch = X.shape[0]
    counts = np.bincount(idx, minlength=N_HEADS)
    order = np.argsort(idx, kind="stable")
    positions = np.split(order, np.cumsum(counts)[:-1])

    C = max(64, int(-(-int(counts.max()) // 8)) * 8)
    if C not in _NC_CACHE:
        _NC_CACHE[C] = build_nc(C)
    nc = _NC_CACHE[C]

    XW = KI * C
    WW = KI * HID
    NIN = XW + WW + HC + OF

    in_maps = []
    for h in range(N_HEADS):
        pos = positions[h]
        cnt = len(pos)
        xinf = np.zeros((P, NIN), dtype=np.float32)
        if cnt:
            # xin[p, k*C + s] = X[pos[s], k*128 + p]
            xk = X[pos].T.reshape(KI, P, cnt)  # [k, p, s]
            for k in range(KI):
                xinf[:, k * C:k * C + cnt] = xk[k]
        # xin[p, XW + k*HID + hcol] = W1[k*128 + p, hcol]
        w1r = W1[h].reshape(KI, P, HID)
        for k in range(KI):
            xinf[:, XW + k * HID:XW + (k + 1) * HID] = w1r[k]
        xinf[:, XW + WW:XW + WW + HC] = b1[h].reshape(HC, P).T
        xinf[:, XW + WW + HC:] = b2[h].reshape(OF, P).T
        # w2 packed: [blk, p, j*HC*P + hc*P + oc] = W2[hc*128+p, (blk*OB+j)*128+oc]
        w2r = W2[h].reshape(HC, P, OF, P)              # [hc, p, of, oc]
        w2p = np.transpose(w2r, (2, 1, 0, 3))          # [of, p, hc, oc]
        w2p = w2p.reshape(OF // OB, OB, P, HC * P)     # [blk, j, p, hc*oc]
        w2p = np.ascontiguousarray(np.transpose(w2p, (0, 2, 1, 3)))
        w2p = w2p.reshape(OF // OB, P, OB * HC * P)
        in_maps.append({
            "xin": xinf.astype(np_bf16),
            "w2": w2p.astype(np_bf16),
        })

    try:
        res = run_bass_kernel_spmd(nc, in_maps, list(range(N_CORES)))
    except Exception:
        res = run_bass_kernel_spmd(nc, in_maps, list(range(N_CORES)))

    out = np.empty((batch, OUT_F), dtype=np.float32)
    for h in range(N_HEADS):
        pos = positions[h]
        cnt = len(pos)
        if cnt:
            o = np.asarray(res.results[h]["outT"]).astype(np.float32)
            o = o.reshape(OF // 2, P, 2, C)            # [pair, p, half, s]
            o = np.transpose(o, (3, 0, 2, 1))          # [s, pair, half, p]
            o = o.reshape(C, OUT_F)
            out[pos] = o[:cnt]
    return out
